# revision 5
# baseline (speedup 1.0000x reference)
"""Locally-connected conv (per-location weights) + ReLU on 8 Trainium2 cores.

Problem: x (B=64, Cin=64, H=64, W=64), weights (H, W, Cout=64, Cin=64, 3, 3)
  out[r,a,i,j] = relu( sum_{b,c,d} weights[i,j,a,b,c,d] * xpad[r,b,i+c,j+d] )

Sharding: data-parallel over H — core cid owns output rows i in [8*cid, 8*cid+8).
No collectives; pure SPMD with per-core input slices.

Device strategy (v2, weight-streaming):
  The weights are 38 MB/core and used once each; x is 5 MB and reused.  The
  v1 kernel made weights the stationary matmul operand, paying a 64-column
  LDWEIGHTS (~53 ns) for every one of 4608 matmuls — LDW-bound at 257 us.
  Here the roles are swapped:
  - x planes are STATIONARY: xp[t] = [128=(2 planes x Cin), WPAD, B] tiles
    stay resident in SBUF; lhsT = xp[t][:, v, :] so out partitions = batch.
  - weights are the MOVING operand, host-packed in exact consumption order
    wt[66, 128, 2304] so each matmul's rhs is a contiguous SBUF column run
    and each DMA is a multi-MB contiguous slab (per-partition 4.6KB runs).
  - Per padded column v and x-tile residency, up to 3 column taps (d=0..2)
    merge into one N<=192 matmul; per-location accumulation (6 contributions)
    happens entirely in PSUM via has_written start/stop groups.
  - Output rows 0-3 use PE column-group 0 (psum partitions 0:64), rows 4-7
    col-group 1 (64:128): two rows share a PSUM bank, so a j-block of 8
    locations needs 4 banks and double-buffers in the other 4.
  - Drain is one DVE op per bank (relu + cast to bf16), then a 512KB DMA.
"""

import ml_dtypes
import numpy as np

import concourse.bass as bass
import concourse.mybir as mybir
import concourse.tile as tile
from concourse import bacc
from concourse.bass_utils import run_bass_kernel_spmd

B = 64          # batch (matmul M: out partitions)
CIN = 64
COUT = 64       # matmul N granularity (64 per location chunk)
H = 64
W = 64
KS = 3
NCORES = 8
RPC = H // NCORES        # output rows per core = 8
NPLANES = RPC + 2        # padded input planes per core = 10
NXT = NPLANES // 2       # paired x tiles = 5
WPAD = W + 2             # 66
NSTREAM = 12             # 8 dual streams (one per row) + 4 single-pair streams
CH = NSTREAM * KS * COUT  # w columns per v = 2304
FP32 = mybir.dt.float32
CDT = mybir.dt.bfloat16
NP_CDT = ml_dtypes.bfloat16

# w DMA slabs (v ranges). First slabs small so the first matmuls start early.
SLABS = [(0, 2), (2, 6)] + [(v, v + 6) for v in range(6, 66, 6)]
MAXSV = 6
# x DMA column chunks (per-partition contiguous 2.8KB runs)
XCHUNKS = [(0, 22), (22, 44), (44, 66)]

# Residency order alternates PE column groups so the next residency's
# LDWEIGHTS lands in idle array columns while the current one streams.
#   (tile t, colgroup, [dual rows], [(kind, row, pairq)])
RESIDENCIES = [
    (0, 0, [0], [("up", 1, 0)]),
    (2, 1, [4], [("up", 5, 2)]),
    (1, 0, [1, 2], [("low", 0, 0), ("up", 3, 1)]),
    (3, 1, [5, 6], [("low", 4, 2), ("up", 7, 3)]),
    (2, 0, [3], [("low", 2, 1)]),
    (4, 1, [7], [("low", 6, 3)]),
]

_PROGRAM = None
LAST_RESULTS = None


def _gen_runs():
    """Yield matmul descriptors in program order.

    Each run: (v, row, tile, kind, stream, dd0, ndd, block)
      kind: 'dual' (K=128), 'low' (K=64, partitions 0:64),
            'up' (K=64, partitions 64:128)
    """
    for v in range(WPAD):
        for (t, cg, duals, singles) in RESIDENCIES:
            items = [("dual", row, t, row) for row in duals] + [
                (kind, row, t, 8 + q) for (kind, row, q) in singles]
            for kind, row, tt, stream in items:
                # valid taps: j = v-2+dd in [0, W)
                dds = [dd for dd in range(KS) if 0 <= v - 2 + dd < W]
                if not dds:
                    continue
                # split runs at j-block boundaries (psum bank = 8 locations)
                run = []
                for dd in dds:
                    j = v - 2 + dd
                    b = j // 8
                    if run and run[-1][1] == b:
                        run[-1][2] += 1
                    else:
                        run.append([dd, b, 1])
                for dd0, b, ndd in run:
                    yield (v, row, tt, kind, stream, dd0, ndd, b)


def _build_program():
    nc = bacc.Bacc("TRN2", target_bir_lowering=False, debug=False,
                   num_devices=NCORES)
    # wt[v, k, c]: k = contraction partition, c = (stream, dd, a) per v.
    wt = nc.dram_tensor("wt", [WPAD, 128, CH], CDT, kind="ExternalInput")
    # xt[plane, b, v, r] — padded x planes for this core's rows.
    xt = nc.dram_tensor("xt", [NPLANES, CIN, WPAD, B], CDT,
                        kind="ExternalInput")
    # ot[jblock, part, cols]: part = (half row r | half row+4 r),
    # cols = pair*512 + jl*64 + a
    ot = nc.dram_tensor("ot", [RPC, 128, 4 * 512], CDT, kind="ExternalOutput")

    # start/stop flags per accumulation group (jblock, row)
    runs = list(_gen_runs())
    first_of = {}
    last_of = {}
    for idx, (v, row, t, kind, stream, dd0, ndd, blk) in enumerate(runs):
        key = (blk, row)
        first_of.setdefault(key, idx)
        last_of[key] = idx

    with tile.TileContext(nc) as tc:
        with (
            tc.tile_pool(name="xpool", bufs=1) as xpool,
            tc.tile_pool(name="wpool", bufs=3) as wpool,
            tc.tile_pool(name="opool", bufs=2) as opool,
            tc.tile_pool(name="pspool", bufs=2,
                         space=bass.MemorySpace.PSUM) as pspool,
        ):
            wtiles = {}

            def get_wtile(s):
                if s not in wtiles and s < len(SLABS):
                    t = wpool.tile([128, MAXSV, CH], CDT, tag="w")
                    v0, v1 = SLABS[s]
                    nc.sync.dma_start(
                        t[:, :v1 - v0, :],
                        wt[v0:v1].rearrange("v k c -> k v c"))
                    wtiles[s] = t
                return wtiles.get(s)

            get_wtile(0)
            xp = []
            for s in range(NXT):
                t = xpool.tile([128, WPAD, B], CDT, tag=f"xp{s}")
                c0, c1 = XCHUNKS[0]
                nc.sync.dma_start(
                    t[:, c0:c1, :],
                    xt[2 * s:2 * s + 2, :, c0:c1, :].rearrange(
                        "p b v r -> (p b) v r"))
                xp.append(t)
            get_wtile(1)
            for c0, c1 in XCHUNKS[1:]:
                for s in range(NXT):
                    nc.sync.dma_start(
                        xp[s][:, c0:c1, :],
                        xt[2 * s:2 * s + 2, :, c0:c1, :].rearrange(
                            "p b v r -> (p b) v r"))

            ps_tiles = {}   # (blk, pair) -> psum tile [128, 512]
            slab_idx = 0
            idx = 0
            for v in range(WPAD):
                # advance w slab; prefetch next
                v0, v1 = SLABS[slab_idx]
                if v >= v1:
                    slab_idx += 1
                    v0, v1 = SLABS[slab_idx]
                wtile = get_wtile(slab_idx)
                get_wtile(slab_idx + 1)
                vs = v - v0

                for (t, cg, duals, singles) in RESIDENCIES:
                    items = [("dual", row, t, row) for row in duals] + [
                        (kind, row, t, 8 + q) for (kind, row, q) in singles]
                    for kind, row, tt, stream in items:
                        dds = [dd for dd in range(KS) if 0 <= v - 2 + dd < W]
                        if not dds:
                            continue
                        run = []
                        for dd in dds:
                            j = v - 2 + dd
                            b = j // 8
                            if run and run[-1][1] == b:
                                run[-1][2] += 1
                            else:
                                run.append([dd, b, 1])
                        for dd0, blk, ndd in run:
                            assert runs[idx] == (v, row, tt, kind, stream,
                                                 dd0, ndd, blk), (
                                runs[idx], (v, row, tt, kind, stream, dd0,
                                            ndd, blk))
                            pair = row % 4
                            pkey = (blk, pair)
                            if pkey not in ps_tiles:
                                ps_tiles[pkey] = pspool.tile(
                                    [128, 512], FP32, tag=f"ps{pair}",
                                    name=f"ps{pair}")
                            ps = ps_tiles[pkey]
                            pbase = 0 if row < 4 else 64
                            jl0 = (v - 2 + dd0) - 8 * blk
                            n = ndd * COUT
                            out_ap = ps[pbase:pbase + 64,
                                        jl0 * 64:jl0 * 64 + n]
                            rcol = stream * (KS * COUT) + dd0 * COUT
                            if kind == "dual":
                                lhsT = xp[tt][:, v, :]
                                rhs = wtile[:, vs, rcol:rcol + n]
                            elif kind == "low":
                                lhsT = xp[tt][0:64, v, :]
                                rhs = wtile[0:64, vs, rcol:rcol + n]
                            else:
                                lhsT = xp[tt][64:128, v, :]
                                rhs = wtile[64:128, vs, rcol:rcol + n]
                            key = (blk, row)
                            nc.tensor.matmul(
                                out_ap, lhsT, rhs,
                                start=(first_of[key] == idx),
                                stop=(last_of[key] == idx))
                            idx += 1

                # block b fully accumulated after v = 8b+9
                if v >= 9 and (v - 9) % 8 == 0:
                    blk = (v - 9) // 8
                    ob = opool.tile([128, 4 * 512], CDT, tag="ob")
                    for pair in range(4):
                        ps = ps_tiles.pop((blk, pair))
                        nc.vector.tensor_scalar_max(
                            ob[:, pair * 512:(pair + 1) * 512], ps[:], 0.0)
                    nc.sync.dma_start(ot[blk], ob[:])
            assert idx == len(runs)
    nc.compile()
    return nc


def _pack_weights(weights):
    """weights (i, j, a, b, c, d) -> Wp[core, v, k, stream, dd, a] bf16.

    Streams 0-7: dual row X (K=128); even X: k=c*64+b c in {0,1};
    odd X: k=(c-1)*64+b c in {1,2}. Streams 8+q: single pair
    (row 2q c=2 at k=b | row 2q+1 c=0 at k=64+b). Chunk (v, dd) holds
    tap d=2-dd for location j=v-2+dd.
    """
    Wp = np.zeros((NCORES, WPAD, 128, NSTREAM, KS, COUT), dtype=NP_CDT)

    def put(core, i, c, dstream, klo):
        # w[i, j, a, b, c, d] -> [j, b, a] per (c, d); place at all dd
        for dd in range(KS):
            d = 2 - dd
            blkw = weights[i, :, :, :, c, d].transpose(0, 2, 1)  # [j, b, a]
            Wp[core, 2 - dd:WPAD - dd, klo:klo + 64, dstream, dd, :] = blkw

    for core in range(NCORES):
        i0 = RPC * core
        for X in range(RPC):
            i = i0 + X
            if X % 2 == 0:
                put(core, i, 0, X, 0)
                put(core, i, 1, X, 64)
            else:
                put(core, i, 1, X, 0)
                put(core, i, 2, X, 64)
        for q in range(4):
            put(core, i0 + 2 * q, 2, 8 + q, 0)
            put(core, i0 + 2 * q + 1, 0, 8 + q, 64)
    return Wp.reshape(NCORES, WPAD, 128, CH)


def _prep_x(x):
    xpad = np.pad(x, ((0, 0), (0, 0), (1, 1), (1, 1)))
    return np.ascontiguousarray(xpad.transpose(2, 1, 3, 0))  # [u, b, v, r]


def kernel(x, weights):
    global _PROGRAM, LAST_RESULTS
    x = np.ascontiguousarray(np.asarray(x, dtype=np.float32))
    weights = np.ascontiguousarray(np.asarray(weights, dtype=np.float32))
    assert x.shape == (B, CIN, H, W)
    assert weights.shape == (H, W, COUT, CIN, KS, KS)

    x_t = _prep_x(x).astype(NP_CDT)
    wp = _pack_weights(weights)

    in_maps = []
    for cid in range(NCORES):
        in_maps.append({
            "wt": np.ascontiguousarray(wp[cid]),
            "xt": np.ascontiguousarray(x_t[RPC * cid:RPC * cid + NPLANES]),
        })

    if _PROGRAM is None:
        _PROGRAM = _build_program()
    res = run_bass_kernel_spmd(_PROGRAM, in_maps, list(range(NCORES)))
    LAST_RESULTS = res

    # ot[jb, part, pair*512 + jl*64 + a] -> out[r, a, i, j]
    out = np.empty((B, COUT, H, W), dtype=np.float32)
    for cid in range(NCORES):
        arr = np.asarray(res.results[cid]["ot"], dtype=np.float32)
        arr = arr.reshape(RPC, 2, 64, 4, 8, 64)      # [jb, half, r, p, jl, a]
        arr = arr.transpose(2, 5, 1, 3, 0, 4)        # [r, a, half, p, jb, jl]
        out[:, :, RPC * cid:RPC * cid + RPC, :] = arr.reshape(B, COUT, RPC, W)
    return out
